# revision 11
# baseline (speedup 1.0000x reference)
"""DBRX attention block (QKV proj + clamp + RoPE + GQA causal attention + out
proj) as a Bass/Tile kernel for 8 Trainium2 NeuronCores.

Problem shapes (hardcoded): B=2, S=2048, HID=2048, NH=16 q-heads, NKV=4 kv
heads, HD=128, clip +-8, rope theta 5e5.

Sharding: DP2 x TP4. Core c = (b = c//4, g = c%4) handles batch b with q-heads
4g..4g+3 and kv-head g (GQA group == core, so no KV duplication). Each core
computes a partial output [S, HID] = attn_flat[:, 512 cols] @ WoutT[512 rows];
the host sums the 4 partials per batch (tensor-parallel reduction).

Everything is computed transposed (qkv^T [o, t] layout): head-dim lands on
partitions, which is what RoPE and the S^T = K^T.T @ Q^T scores matmul want.
All matmul operands are bf16 (PSUM accumulation stays f32): same 1 cycle/row
PE throughput as f32r but half the DMA/SBUF traffic and 2x DVE modes.

The qkv projection and attention are interleaved per 512-token block: after
the qkv+RoPE of t-block tb, the attention q-block j=tb (which only needs K/V
up to that point, by causality) is emitted. This spreads the ScalarE exp work
(the attention phase's co-bottleneck) across the whole kernel instead of
bunching it after the projection phase.
"""

import math
from contextlib import ExitStack

import numpy as np

import concourse.bacc as bacc
import concourse.bass as bass
import concourse.mybir as mybir
import concourse.tile as tile
from concourse.bass_utils import run_bass_kernel_spmd
from concourse.masks import make_identity

P = 128
B, S, HID = 2, 2048, 2048
NH, NKV, HD = 16, 4, 128
CLIP = 8.0
ROPE_THETA = 500000.0
NQ = NH // NKV        # q heads per core = 4
OC = NQ + 2           # o-chunks per core: 4 q heads, 1 k, 1 v
KC = HID // P         # 16 contraction chunks for qkv proj
TB = 512              # t-block (moving free dim) for qkv proj
NTB = S // TB         # 4
QB = 512              # q-block in attention (== TB so phases interleave 1:1)
NCORES = 8

BF = mybir.dt.bfloat16
F32 = mybir.dt.float32


def build_nc(reps: int = 1):
    nc = bacc.Bacc()

    hT = nc.dram_tensor("hT", [HID, S], BF, kind="ExternalInput")
    wqkvT = nc.dram_tensor("wqkvT", [HID, OC * P], BF, kind="ExternalInput")
    woutT = nc.dram_tensor("woutT", [NQ * P, HID], BF, kind="ExternalInput")
    cosT = nc.dram_tensor("cosT", [P, S], BF, kind="ExternalInput")
    sinT = nc.dram_tensor("sinT", [P, S], BF, kind="ExternalInput")
    out = nc.dram_tensor("out", [S, HID], BF, kind="ExternalOutput")

    hT3 = hT.rearrange("(kc p) t -> p kc t", p=P)          # [128, 16, 2048]
    wq3 = wqkvT.rearrange("(kc p) o -> p kc o", p=P)       # [128, 16, 768]
    wo3 = woutT.rearrange("(c p) o -> p c o", p=P)         # [128, 4, 2048]
    out3 = out.rearrange("(tc p) o -> p tc o", p=P)        # [128, 16, 2048]

    with TileCtx(nc, reps) as tc:
        emit_body(nc, tc, hT3, wq3, wo3, cosT, sinT, out3)

    nc.compile()
    return nc


class TileCtx:
    """TileContext wrapper that optionally wraps the body in a repeat loop
    (used only for wall-clock timing; the graded build uses reps=1)."""

    def __init__(self, nc, reps):
        self.nc = nc
        self.reps = reps
        self.tc = tile.TileContext(nc)
        self.loop = None

    def __enter__(self):
        tc = self.tc.__enter__()
        if self.reps > 1:
            self.loop = tc.For_i(0, self.reps, 1)
            self.loop.__enter__()
        return tc

    def __exit__(self, *a):
        if self.loop is not None:
            self.loop.__exit__(*a)
        return self.tc.__exit__(*a)


def emit_body(nc, tc, hT3, wq3, wo3, cosT, sinT, out3):
    inv_sqrt_hd = 1.0 / math.sqrt(HD)
    with ExitStack() as ctx:
        persist = ctx.enter_context(tc.tile_pool(name="persist", bufs=1))
        qkv = persist.tile([P, OC, S], BF)      # q0..q3, k, v^T  (qkv^T layout)
        V = persist.tile([P, S], BF)            # [t_local, (kb, d)] V blocks
        attnT = persist.tile([P, NQ, S], BF)
        wq_sb = persist.tile([P, KC, OC * P], BF)
        wout_sb = persist.tile([P, NQ, HID], BF)
        cos_sb = persist.tile([P, S], BF)
        sin_sb = persist.tile([P, S], BF)
        ones = persist.tile([P, P], BF)
        rotm = persist.tile([P, P], BF)         # rotate-half permutation (RT)
        ident = persist.tile([P, P], BF)
        masks = persist.tile([P, P], BF)    # lower-triangular-incl. 0/1 mask
        setup_f32 = persist.tile([P, P], F32)

        # ---- one-time constants (cheap; Pool + DVE) ----
        nc.gpsimd.memset(setup_f32, 1.0)
        nc.vector.tensor_copy(out=ones, in_=setup_f32)
        # rotm[p, x] = 1 at x = (p+64) % 128: lhsT of the rotate-half matmul
        nc.gpsimd.memset(setup_f32, 0.0)
        nc.gpsimd.affine_select(   # +1 at x = p + 64 (p < 64)
            out=setup_f32, in_=setup_f32,
            compare_op=mybir.AluOpType.not_equal, fill=1.0,
            base=64, channel_multiplier=1, pattern=[[-1, P]])
        nc.gpsimd.affine_select(   # +1 at x = p - 64 (p >= 64)
            out=setup_f32, in_=setup_f32,
            compare_op=mybir.AluOpType.not_equal, fill=1.0,
            base=-64, channel_multiplier=1, pattern=[[-1, P]])
        nc.vector.tensor_copy(out=rotm, in_=setup_f32)
        make_identity(nc, setup_f32)
        nc.vector.tensor_copy(out=ident, in_=setup_f32)
        nc.gpsimd.memset(setup_f32, 1.0)
        # keep 1.0 where q_local >= k_local, else 0
        nc.gpsimd.affine_select(
            out=setup_f32, in_=setup_f32,
            compare_op=mybir.AluOpType.is_ge, fill=0.0,
            base=0, channel_multiplier=-1, pattern=[[1, P]])
        nc.vector.tensor_copy(out=masks, in_=setup_f32)

        # ---- input DMAs ----
        # tb0 weights + activations interleaved per kc so the first
        # accumulation group's matmuls start after ~1 chunk. h on the SP
        # queue; weights/tables on the Pool queue; outputs on the Act queue.
        h_pool = ctx.enter_context(tc.tile_pool(name="ht", bufs=2))
        h_t0 = h_pool.tile([P, KC, TB], BF, tag="ht")
        for kc in range(KC):
            nc.gpsimd.dma_start(out=wq_sb[:, kc, NQ * P:],
                                in_=wq3[:, kc, NQ * P:])    # k+v cols
            nc.sync.dma_start(out=h_t0[:, kc, :], in_=hT3[:, kc, 0:TB])
            nc.gpsimd.dma_start(out=wq_sb[:, kc, :NQ * P],
                                in_=wq3[:, kc, :NQ * P])    # q cols
            if kc == 0:
                nc.gpsimd.dma_start(out=cos_sb, in_=cosT[:, :])
                nc.gpsimd.dma_start(out=sin_sb, in_=sinT[:, :])
        nc.gpsimd.dma_start(out=wout_sb, in_=wo3)

        # PSUM pools: 2 + 3 + 2 + 1 = 8 banks. Closed before phase 3 so the
        # out-proj pool can take the whole PSUM.
        p12 = ctx.enter_context(ExitStack())
        psA = p12.enter_context(tc.tile_pool(name="psA", bufs=1, space="PSUM"))
        psS = p12.enter_context(tc.tile_pool(name="psS", bufs=3, space="PSUM"))
        psO = p12.enter_context(tc.tile_pool(name="psO", bufs=2, space="PSUM"))
        psD = p12.enter_context(tc.tile_pool(name="psD", bufs=1, space="PSUM"))
        rope_p = p12.enter_context(tc.tile_pool(name="rope", bufs=2))
        p_pool = p12.enter_context(tc.tile_pool(name="pp", bufs=3))
        nrm_pool = p12.enter_context(tc.tile_pool(name="nrm", bufs=2))

        def rope_block(oc, tb):
            sl = slice(tb * TB, (tb + 1) * TB)
            ch = qkv[:, oc, sl]
            rps = psS.tile([P, TB], F32, tag="s")
            nc.tensor.matmul(rps, rotm, ch, start=True, stop=True)
            t1 = rope_p.tile([P, TB], BF, tag="t1")
            nc.vector.tensor_mul(t1, rps, sin_sb[:, sl])
            t2 = rope_p.tile([P, TB], BF, tag="t2")
            nc.gpsimd.tensor_mul(t2, ch, cos_sb[:, sl])
            nc.vector.tensor_add(ch, t1, t2)

        kT = qkv[:, NQ, :]
        for tb in range(NTB):
            if tb == 0:
                h_t = h_t0
            else:
                h_t = h_pool.tile([P, KC, TB], BF, tag="ht")
                nc.sync.dma_start(
                    out=h_t, in_=hT3[:, :, tb * TB:(tb + 1) * TB])
            # ---- qkv^T = WqkvT.T @ hT + clip (+ RoPE / V transpose) ----
            for oc in [NQ, 0, 1, 2, 3, NQ + 1]:   # k first, then qs, v
                ps = psA.tile([P, TB], F32, tag="qp", bufs=2)
                for kc in range(KC):
                    nc.tensor.matmul(
                        ps,
                        wq_sb[:, kc, oc * P:(oc + 1) * P],
                        h_t[:, kc, :],
                        start=(kc == 0),
                        stop=(kc == KC - 1),
                    )
                # clip(x) = max(min(x, 8), -8), round to bf16
                nc.vector.tensor_scalar(
                    out=qkv[:, oc, tb * TB:(tb + 1) * TB],
                    in0=ps,
                    scalar1=CLIP,
                    scalar2=-CLIP,
                    op0=mybir.AluOpType.min,
                    op1=mybir.AluOpType.max,
                )
                if oc <= NQ:
                    rope_block(oc, tb)
                else:
                    # v chunk: transpose each 128-block into V [t_loc,(kb,d)]
                    vps = psD.tile([P, TB], BF, tag="d")
                    for i in range(TB // P):
                        blk = qkv[:, NQ + 1,
                                  (tb * (TB // P) + i) * P:
                                  (tb * (TB // P) + i + 1) * P]
                        nc.tensor.transpose(
                            vps[:, i * P:(i + 1) * P], blk, ident)
                    nc.vector.tensor_copy(
                        out=V[:, tb * TB:(tb + 1) * TB], in_=vps)

            # ---- attention q-block j = tb (all heads), causal GQA, S^T ----
            j = tb
            nk = (j + 1) * (QB // P)     # causal: k-blocks 0..nk-1
            for h in range(NQ):
                qT = qkv[:, h, :]
                ps_o = psO.tile([P, QB], F32, tag="o")
                ps_d = psD.tile([P, QB], F32, tag="d")
                ps_s_t = {}

                def geom(kb):
                    # diagonal blocks (r>=0): cols < 128r are fully masked -
                    # skip them; the first 128 live cols are triangular, the
                    # rest fully valid.
                    r = kb - j * (QB // P)
                    q0 = max(0, P * r)       # first live col in q-block
                    return r, q0, QB - q0

                def emit_scores(kb):
                    r, q0, w = geom(kb)
                    ps_s = psS.tile([P, QB], F32, tag="s")
                    nc.tensor.matmul(
                        ps_s[:, :w],
                        kT[:, kb * P:(kb + 1) * P],
                        qT[:, j * QB + q0:(j + 1) * QB],
                        start=True, stop=True,
                    )
                    ps_s_t[kb] = ps_s

                # software pipeline: keep scores 2 blocks ahead of PV so the
                # PE never waits on the ScalarE exp of the current block.
                emit_scores(0)
                if nk > 1:
                    emit_scores(1)
                for kb in range(nk):
                    r, q0, w = geom(kb)
                    ps_s = ps_s_t.pop(kb)
                    p_t = p_pool.tile([P, QB], BF, tag="pt")
                    nc.scalar.activation(
                        p_t[:, :w], ps_s[:, :w],
                        mybir.ActivationFunctionType.Exp,
                        scale=inv_sqrt_hd)
                    if r >= 0:               # triangular 128-col head
                        nc.vector.tensor_mul(p_t[:, :P], p_t[:, :P], masks)
                    nc.tensor.matmul(
                        ps_o[:, q0:], V[:, kb * P:(kb + 1) * P], p_t[:, :w],
                        start=(kb == 0), stop=(kb == nk - 1))
                    nc.tensor.matmul(
                        ps_d[:, q0:], ones, p_t[:, :w],
                        start=(kb == 0), stop=(kb == nk - 1))
                    if kb + 2 < nk:
                        emit_scores(kb + 2)
                recip = nrm_pool.tile([P, QB], F32, tag="recip")
                nc.vector.reciprocal(recip, ps_d)
                nc.vector.tensor_mul(
                    attnT[:, h, j * QB:(j + 1) * QB], ps_o, recip)

        # ---------------- phase 3: out = attn_flat @ WoutT -------------------
        p12.close()
        with ExitStack() as p4:
            outp = p4.enter_context(tc.tile_pool(name="outp", bufs=3))
            ps_f_pool = p4.enter_context(
                tc.tile_pool(name="psf", bufs=6, space="PSUM"))
            for tci in range(S // P):
                o_row = outp.tile([P, HID], BF, tag="orow")
                for ob in range(HID // TB):
                    ps = ps_f_pool.tile([P, TB], F32)
                    for c in range(NQ):
                        nc.tensor.matmul(
                            ps,
                            attnT[:, c, tci * P:(tci + 1) * P],
                            wout_sb[:, c, ob * TB:(ob + 1) * TB],
                            start=(c == 0), stop=(c == NQ - 1),
                        )
                    if ob % 2 == 0:
                        nc.scalar.copy(
                            out=o_row[:, ob * TB:(ob + 1) * TB], in_=ps)
                    else:
                        nc.vector.tensor_copy(
                            out=o_row[:, ob * TB:(ob + 1) * TB], in_=ps)
                eng = nc.scalar if tci % 2 == 0 else nc.sync
                eng.dma_start(out=out3[:, tci, :], in_=o_row)


def prepare_inputs(hidden_states, position_ids, Wqkv, Wout):
    import ml_dtypes
    bf16 = ml_dtypes.bfloat16

    hidden_states = np.asarray(hidden_states, dtype=np.float32)
    position_ids = np.asarray(position_ids)
    Wqkv = np.asarray(Wqkv, dtype=np.float32)
    Wout = np.asarray(Wout, dtype=np.float32)

    # rope tables, mirroring the reference's f32 math
    inv_freq = (1.0 / (ROPE_THETA ** (np.arange(0, HD, 2, dtype=np.float32)
                                      / np.float32(HD)))).astype(np.float32)
    in_maps = []
    for c in range(NCORES):
        b, g = divmod(c, NQ)
        pos = position_ids[b].astype(np.float32)
        freqs = pos[:, None] * inv_freq[None, :]            # [S, 64] f32
        cos = np.cos(np.concatenate([freqs, freqs], axis=1))  # [S, 128]
        sin = np.sin(np.concatenate([freqs, freqs], axis=1))
        sinS = sin.T.copy()                                  # [128, S]
        sinS[:HD // 2] *= -1.0                               # rotate-half sign
        wq_rows = np.concatenate([
            Wqkv[512 * g:512 * (g + 1)],                     # 4 q heads
            Wqkv[NH * HD + HD * g: NH * HD + HD * (g + 1)],  # k head
            Wqkv[(NH + NKV) * HD + HD * g:
                 (NH + NKV) * HD + HD * (g + 1)],            # v head
        ], axis=0)                                           # [768, HID]
        in_maps.append({
            "hT": np.ascontiguousarray(hidden_states[b].T).astype(bf16),
            "wqkvT": np.ascontiguousarray(wq_rows.T).astype(bf16),
            "woutT": np.ascontiguousarray(
                Wout[:, 512 * g:512 * (g + 1)].T).astype(bf16),
            "cosT": np.ascontiguousarray(cos.T).astype(bf16),
            "sinT": np.ascontiguousarray(sinS).astype(bf16),
        })
    return in_maps


def assemble(results):
    out = np.zeros((B, S, HID), dtype=np.float64)
    for c in range(NCORES):
        b = c // NQ
        out[b] += results[c]["out"].astype(np.float64)
    return out.astype(np.float32)


_cache = {}


def kernel(hidden_states, position_ids, Wqkv, Wout):
    if "nc" not in _cache:
        _cache["nc"] = build_nc(reps=1)
    nc = _cache["nc"]
    in_maps = prepare_inputs(hidden_states, position_ids, Wqkv, Wout)
    res = run_bass_kernel_spmd(nc, in_maps, core_ids=list(range(NCORES)))
    return assemble(res.results)
